# revision 6
# baseline (speedup 1.0000x reference)
"""Multi-head causal attention (B=2, T=2048, C=1024, H=16, D=64) on 8 TRN2
NeuronCores, tensor-parallel over heads: core c owns heads {2c, 2c+1}.

Per-core program (SPMD, different weight slices per core):
  qkv^T = W_c^T @ x^T + b_c            [384, 4096]  (fp32r matmuls, K=128 chunks)
    rows   0-127 : Q^T (h0 d at partitions 0-63, h1 d at 64-127)
    rows 128-255 : K^T
    rows 256-383 : V^T
  V natural [k, d] via PE transpose (+ ones column for the denominator trick)
  per (b, head, q-chunk of 512):
    s^T[k, q] = K^T.T @ Q^T  (K=64 contraction), exp via ACT (causal-shortened),
    causal zeroing via gpsimd affine_select,
    out^T[d, q] (+ denom row) = [V | 1].T @ P^T  (K=128 accumulation),
    PE-transpose back to [q, d], scale by 1/denom, assemble [q, 128] c-slice.
Host: transposes x once (layout prep), slices W/b per core, concatenates the
per-core [4096, 128] outputs along the channel dim.
"""
import sys

if "/opt/trn_rl_repo" not in sys.path:
    sys.path.insert(0, "/opt/trn_rl_repo")

from contextlib import ExitStack

import numpy as np

import concourse.bass as bass
import concourse.tile as tile
from concourse import bacc, mybir
from concourse._compat import with_exitstack
from concourse.bass_utils import run_bass_kernel_spmd
from concourse.masks import make_identity

F32 = mybir.dt.float32
F32R = mybir.dt.float32r
EXPF = mybir.ActivationFunctionType.Exp

B, T, C = 2, 2048, 1024
H, D = 16, 64
NCORES = 8
TOK = B * T            # 4096
CS = 128               # channel slice per core (2 heads x 64)
N3 = 3 * CS            # 384 qkv columns per core
SCALE = 1.0 / np.sqrt(D)


@with_exitstack
def mha_kernel(ctx: ExitStack, tc: tile.TileContext, out_ap, xT_ap, w_ap, b_ap):
    nc = tc.nc

    cst = ctx.enter_context(tc.tile_pool(name="cst", bufs=1))
    xpool = ctx.enter_context(tc.tile_pool(name="xt", bufs=10))
    ppool = ctx.enter_context(tc.tile_pool(name="pt", bufs=18))
    otpool = ctx.enter_context(tc.tile_pool(name="ot", bufs=2))
    rpool = ctx.enter_context(tc.tile_pool(name="rc", bufs=2))
    ps_proj = ctx.enter_context(tc.tile_pool(name="ps_proj", bufs=2, space="PSUM"))
    ps_s = ctx.enter_context(tc.tile_pool(name="ps_s", bufs=2, space="PSUM"))
    ps_pv = ctx.enter_context(tc.tile_pool(name="ps_pv", bufs=1, space="PSUM"))
    ps_t = ctx.enter_context(tc.tile_pool(name="ps_t", bufs=1, space="PSUM"))

    ident_f = cst.tile([128, 128], F32, name="ident_f")
    make_identity(nc, ident_f[:])
    ident_r = cst.tile([128, 128], F32R, name="ident_r")
    nc.vector.tensor_copy(ident_r[:], ident_f[:])

    # bias cols 0-2; cols 3-18 are ones (for the PV denominator column)
    bias = cst.tile([128, 19], F32, name="bias")
    nc.sync.dma_start(bias[:], b_ap[:])

    W8 = []
    for cc in range(8):
        w = cst.tile([128, N3], F32R, name=f"w{cc}")
        nc.sync.dma_start(w[:], w_ap[128 * cc : 128 * (cc + 1), :].bitcast(F32R))
        W8.append(w)

    qkvT = [cst.tile([128, TOK], F32R, name=f"qkvT{nt}") for nt in range(3)]

    # ---- projection: qkvT[nt][:, t0:t0+512] = W[:, nt].T @ xT[:, t] + b ----
    for tq in range(4):  # t-quarters of 1024 tokens
        xts = []
        for cc in range(8):
            xt = xpool.tile([128, 1024], F32R, name="xt")
            nc.sync.dma_start(
                xt[:],
                xT_ap[128 * cc : 128 * (cc + 1), 1024 * tq : 1024 * (tq + 1)].bitcast(F32R),
            )
            xts.append(xt)
        for th in range(2):
            t0 = 1024 * tq + 512 * th
            for nt in range(3):
                ps = ps_proj.tile([128, 512], F32, name="ps_proj")
                for cc in range(8):
                    nc.tensor.matmul(
                        ps[:],
                        lhsT=W8[cc][:, 128 * nt : 128 * (nt + 1)],
                        rhs=xts[cc][:, 512 * th : 512 * (th + 1)],
                        start=(cc == 0),
                        stop=(cc == 7),
                    )
                nc.vector.tensor_scalar_add(
                    qkvT[nt][:, t0 : t0 + 512], ps[:], bias[:, nt : nt + 1]
                )

    # ---- V natural [k, d] + ones column, per (b, hh) ----
    V1 = []
    for b in range(B):
        for hh in range(2):
            v1 = cst.tile([128, 16 * 65], F32R, name=f"v1_{b}_{hh}")
            v3 = v1[:].rearrange("p (k u) -> p k u", u=65)
            nc.vector.tensor_copy(v3[:, :, 64], bias[:, 3:19])
            for g in range(2):
                tp = ps_t.tile([128, 512], F32R, name="ps_vt")
                for j in range(8):
                    kc = 8 * g + j
                    nc.tensor.transpose(
                        tp[:, 64 * j : 64 * (j + 1)],
                        qkvT[2][64 * hh : 64 * (hh + 1), 2048 * b + 128 * kc : 2048 * b + 128 * (kc + 1)],
                        ident_r[64 * hh : 64 * hh + 64, 64 * hh : 64 * hh + 64],
                    )
                nc.vector.tensor_copy(
                    v3[:, 8 * g : 8 * (g + 1), 0:64],
                    tp[:].rearrange("p (k u) -> p k u", u=64),
                )
            V1.append(v1)

    # ---- attention ----
    outsb = [cst.tile([128, 2048], F32, name=f"outsb{b}") for b in range(B)]

    for b in range(B):
        for hh in range(2):
            qh = qkvT[0][64 * hh : 64 * (hh + 1), :]
            kh = qkvT[1][64 * hh : 64 * (hh + 1), :]
            v3 = V1[2 * b + hh][:].rearrange("p (k u) -> p k u", u=65)
            for qc in range(4):
                q0 = 2048 * b + 512 * qc
                nk = 4 * qc + 4
                ptiles = []
                for ki in range(nk):
                    k0 = 2048 * b + 128 * ki
                    sp = ps_s.tile([128, 512], F32, name="ps_s")
                    nc.tensor.matmul(
                        sp[:],
                        lhsT=kh[:, k0 : k0 + 128],
                        rhs=qh[:, q0 : q0 + 512],
                        start=True,
                        stop=True,
                    )
                    P = ppool.tile([128, 512], F32R, name="P")
                    r = 128 * ki - 512 * qc
                    if r < 0:
                        nc.scalar.activation(P[:], sp[:], EXPF, scale=SCALE)
                    else:
                        nc.scalar.activation(P[:, r:512], sp[:, r:512], EXPF, scale=SCALE)
                        nc.gpsimd.affine_select(
                            out=P[:, r:512],
                            in_=P[:, r:512],
                            compare_op=mybir.AluOpType.is_ge,
                            fill=0.0,
                            base=0,
                            pattern=[[1, 512 - r]],
                            channel_multiplier=-1,
                        )
                    ptiles.append(P)
                op = ps_pv.tile([65, 512], F32, name="ps_pv")
                for ki in range(nk):
                    r = max(0, 128 * ki - 512 * qc)
                    nc.tensor.matmul(
                        op[:, r:512],
                        lhsT=v3[:, ki, :],
                        rhs=ptiles[ki][:, r:512],
                        start=(ki == 0),
                        stop=(ki == nk - 1),
                    )
                oT = otpool.tile([65, 512], F32, name="oT")
                nc.vector.tensor_copy(oT[:], op[:])
                dp = ps_t.tile([128, 4], F32, name="ps_dp")
                for j in range(4):
                    nc.tensor.transpose(
                        dp[:, j : j + 1],
                        oT[64:65, 128 * j : 128 * (j + 1)],
                        ident_f[64:65, 64:65],
                    )
                rc = rpool.tile([128, 4], F32, name="rc")
                nc.vector.reciprocal(rc[:], dp[:])
                for j in range(4):
                    ot2 = ps_t.tile([128, 64], F32, name="ps_ot2")
                    nc.tensor.transpose(
                        ot2[:],
                        oT[0:64, 128 * j : 128 * (j + 1)],
                        ident_f[0:64, 0:64],
                    )
                    col = 128 * (4 * qc + j) + 64 * hh
                    nc.vector.tensor_scalar_mul(
                        outsb[b][:, col : col + 64], ot2[:], rc[:, j : j + 1]
                    )

    for b in range(B):
        dst = out_ap[2048 * b : 2048 * (b + 1), :].rearrange("(k p) c -> p k c", p=128)
        src = outsb[b][:].rearrange("p (k c) -> p k c", c=128)
        nc.sync.dma_start(dst, src)


def build_program():
    nc = bacc.Bacc("TRN2", target_bir_lowering=False, debug=False, num_devices=NCORES)
    xT_h = nc.dram_tensor("xT", [C, TOK], F32, kind="ExternalInput").ap()
    w_h = nc.dram_tensor("w", [C, N3], F32, kind="ExternalInput").ap()
    b_h = nc.dram_tensor("b", [128, 19], F32, kind="ExternalInput").ap()
    out_h = nc.dram_tensor("out", [TOK, CS], F32, kind="ExternalOutput").ap()
    with tile.TileContext(nc) as tc:
        mha_kernel(tc, out_h, xT_h, w_h, b_h)
    nc.compile()
    return nc


def make_in_maps(x, w_qkv, b_qkv):
    x = np.asarray(x, dtype=np.float32)
    w_qkv = np.asarray(w_qkv, dtype=np.float32)
    b_qkv = np.asarray(b_qkv, dtype=np.float32)
    xT = np.ascontiguousarray(x.reshape(TOK, C).T)  # [C, TOK]
    in_maps = []
    for c in range(NCORES):
        sl = slice(CS * c, CS * (c + 1))
        w_c = np.ascontiguousarray(
            np.concatenate(
                [w_qkv[:, sl], w_qkv[:, C + CS * c : C + CS * (c + 1)],
                 w_qkv[:, 2 * C + CS * c : 2 * C + CS * (c + 1)]],
                axis=1,
            )
        )
        b_c = np.concatenate(
            [b_qkv[sl], b_qkv[C + CS * c : C + CS * (c + 1)],
             b_qkv[2 * C + CS * c : 2 * C + CS * (c + 1)]]
        )
        b_c = b_c.reshape(3, 128).T  # [128, 3]
        b_c = np.ascontiguousarray(
            np.concatenate([b_c, np.ones((128, 16), np.float32)], axis=1)
        )  # [128, 19]
        in_maps.append({"xT": xT, "w": w_c, "b": b_c})
    return in_maps


_NC_CACHE = None


def kernel(x, w_qkv, b_qkv):
    global _NC_CACHE
    if _NC_CACHE is None:
        _NC_CACHE = build_program()
    nc = _NC_CACHE
    in_maps = make_in_maps(x, w_qkv, b_qkv)
    res = run_bass_kernel_spmd(nc, in_maps, list(range(NCORES)))
    outs = [res.results[c]["out"].reshape(B, T, CS) for c in range(NCORES)]
    return np.concatenate(outs, axis=2)


if __name__ == "__main__":
    rng = np.random.default_rng(0)
    x = rng.standard_normal((B, T, C), dtype=np.float32)
    w = (rng.standard_normal((C, 3 * C), dtype=np.float32) / np.sqrt(C)).astype(np.float32)
    bq = (rng.standard_normal((3 * C,), dtype=np.float32) * 0.02).astype(np.float32)
    out = kernel(x, w, bq)
    print("out", out.shape, out.dtype)


# revision 7
# speedup vs baseline: 53.1286x; 53.1286x over previous
"""Multi-head causal attention (B=2, T=2048, C=1024, H=16, D=64) on 8 TRN2
NeuronCores, tensor-parallel over heads: core c owns heads {2c, 2c+1}.

Per-core program (SPMD, same code, per-core weight slices):
  qkv^T = W_c^T @ x^T + b_c          [384, 4096]  (bf16 matmuls, fp32 psum)
    rows   0-127 : Q^T (h0 d at partitions 0-63, h1 at 64-127)
    rows 128-255 : K^T
    rows 256-383 : V^T
  V natural [k, d] via PE transpose, with a ones column appended per k-chunk
  per (b, head, 512-wide q-chunk):
    s^T[k, q] = K^T.T @ Q^T   (K=64, bf16), exp on ACT (causal-shortened),
    causal upper-triangle zeroing via gpsimd affine_select,
    outT[d,q] + denom row = [V | 1].T @ P^T  (K=128 psum accumulation),
    PE-transpose back to [q, d], scale by 1/denom, assemble the [q, 128]
    channel slice, one big DMA out per batch.
Host: transposes x and converts x/W to bf16 (input layout prep), slices
W/b per core, concatenates per-core [4096, 128] outputs on channels.
"""
import sys

if "/opt/trn_rl_repo" not in sys.path:
    sys.path.insert(0, "/opt/trn_rl_repo")

from contextlib import ExitStack

import numpy as np
import ml_dtypes

import concourse.bass as bass
import concourse.tile as tile
from concourse import bacc, mybir
from concourse._compat import with_exitstack
from concourse.bass_utils import run_bass_kernel_spmd
from concourse.masks import make_identity

F32 = mybir.dt.float32
BF16 = mybir.dt.bfloat16
EXPF = mybir.ActivationFunctionType.Exp

B, T, C = 2, 2048, 1024
H, D = 16, 64
NCORES = 8
TOK = B * T            # 4096
CS = 128               # channel slice per core (2 heads x 64)
N3 = 3 * CS            # 384 qkv columns per core
SCALE = 1.0 / np.sqrt(D)


@with_exitstack
def mha_kernel(ctx: ExitStack, tc: tile.TileContext, out_ap, xT_ap, w_ap, b_ap):
    nc = tc.nc

    cst = ctx.enter_context(tc.tile_pool(name="cst", bufs=1))
    xpool = ctx.enter_context(tc.tile_pool(name="xt", bufs=10))
    ppool = ctx.enter_context(tc.tile_pool(name="pt", bufs=18))
    otpool = ctx.enter_context(tc.tile_pool(name="ot", bufs=2))
    rpool = ctx.enter_context(tc.tile_pool(name="rc", bufs=2))
    ps_proj = ctx.enter_context(tc.tile_pool(name="ps_proj", bufs=2, space="PSUM"))
    ps_s = ctx.enter_context(tc.tile_pool(name="ps_s", bufs=2, space="PSUM"))
    ps_pv = ctx.enter_context(tc.tile_pool(name="ps_pv", bufs=1, space="PSUM"))
    ps_t = ctx.enter_context(tc.tile_pool(name="ps_t", bufs=1, space="PSUM"))

    ident_f = cst.tile([128, 128], F32, name="ident_f")
    make_identity(nc, ident_f[:])
    ident_b = cst.tile([128, 128], BF16, name="ident_b")
    nc.vector.tensor_copy(ident_b[:], ident_f[:])

    # bias cols 0-2; cols 3-18 are ones (for the PV denominator column)
    bias = cst.tile([128, 19], F32, name="bias")
    nc.sync.dma_start(bias[:], b_ap[:])

    W8 = []
    for cc in range(8):
        w = cst.tile([128, N3], BF16, name=f"w{cc}")
        nc.sync.dma_start(w[:], w_ap[128 * cc : 128 * (cc + 1), :])
        W8.append(w)

    qkvT = [cst.tile([128, TOK], BF16, name=f"qkvT{nt}") for nt in range(3)]

    # ---- projection: qkvT[nt][:, t0:t0+512] = W[:, nt].T @ xT[:, t] + b ----
    for tq in range(4):  # t-quarters of 1024 tokens
        xts = []
        for cc in range(8):
            xt = xpool.tile([128, 1024], BF16, name="xt")
            nc.sync.dma_start(
                xt[:],
                xT_ap[128 * cc : 128 * (cc + 1), 1024 * tq : 1024 * (tq + 1)],
            )
            xts.append(xt)
        for th in range(2):
            t0 = 1024 * tq + 512 * th
            for nt in range(3):
                ps = ps_proj.tile([128, 512], F32, name="ps_proj")
                for cc in range(8):
                    nc.tensor.matmul(
                        ps[:],
                        lhsT=W8[cc][:, 128 * nt : 128 * (nt + 1)],
                        rhs=xts[cc][:, 512 * th : 512 * (th + 1)],
                        start=(cc == 0),
                        stop=(cc == 7),
                    )
                nc.vector.tensor_scalar_add(
                    qkvT[nt][:, t0 : t0 + 512], ps[:], bias[:, nt : nt + 1]
                )

    # ---- V natural [k, d] + ones column, per (b, hh) ----
    V1 = []
    for b in range(B):
        for hh in range(2):
            v1 = cst.tile([128, 16 * 65], BF16, name=f"v1_{b}_{hh}")
            v3 = v1[:].rearrange("p (k u) -> p k u", u=65)
            nc.vector.tensor_copy(v3[:, :, 64], bias[:, 3:19])
            for g in range(2):
                tp = ps_t.tile([128, 512], BF16, name="ps_vt")
                for j in range(8):
                    kc = 8 * g + j
                    nc.tensor.transpose(
                        tp[:, 64 * j : 64 * (j + 1)],
                        qkvT[2][64 * hh : 64 * (hh + 1), 2048 * b + 128 * kc : 2048 * b + 128 * (kc + 1)],
                        ident_b[64 * hh : 64 * hh + 64, 64 * hh : 64 * hh + 64],
                    )
                nc.vector.tensor_copy(
                    v3[:, 8 * g : 8 * (g + 1), 0:64],
                    tp[:].rearrange("p (k u) -> p k u", u=64),
                )
            V1.append(v1)

    # ---- attention ----
    outsb = [cst.tile([128, 2048], F32, name=f"outsb{b}") for b in range(B)]

    for b in range(B):
        for hh in range(2):
            qh = qkvT[0][64 * hh : 64 * (hh + 1), :]
            kh = qkvT[1][64 * hh : 64 * (hh + 1), :]
            v3 = V1[2 * b + hh][:].rearrange("p (k u) -> p k u", u=65)
            for qc in range(4):
                q0 = 2048 * b + 512 * qc
                nk = 4 * qc + 4
                ptiles = []
                for ki in range(nk):
                    k0 = 2048 * b + 128 * ki
                    sp = ps_s.tile([128, 512], F32, name="ps_s")
                    nc.tensor.matmul(
                        sp[:],
                        lhsT=kh[:, k0 : k0 + 128],
                        rhs=qh[:, q0 : q0 + 512],
                        start=True,
                        stop=True,
                    )
                    P = ppool.tile([128, 512], BF16, name="P")
                    r = 128 * ki - 512 * qc
                    if r < 0:
                        nc.scalar.activation(P[:], sp[:], EXPF, scale=SCALE)
                    else:
                        nc.scalar.activation(P[:, r:512], sp[:, r:512], EXPF, scale=SCALE)
                        nc.gpsimd.affine_select(
                            out=P[:, r:512],
                            in_=P[:, r:512],
                            compare_op=mybir.AluOpType.is_ge,
                            fill=0.0,
                            base=0,
                            pattern=[[1, 512 - r]],
                            channel_multiplier=-1,
                        )
                    ptiles.append(P)
                op = ps_pv.tile([65, 512], F32, name="ps_pv")
                for ki in range(nk):
                    r = max(0, 128 * ki - 512 * qc)
                    nc.tensor.matmul(
                        op[:, r:512],
                        lhsT=v3[:, ki, :],
                        rhs=ptiles[ki][:, r:512],
                        start=(ki == 0),
                        stop=(ki == nk - 1),
                    )
                oT = otpool.tile([65, 512], F32, name="oT")
                nc.vector.tensor_copy(oT[:], op[:])
                dp = ps_t.tile([128, 4], F32, name="ps_dp")
                for j in range(4):
                    nc.tensor.transpose(
                        dp[:, j : j + 1],
                        oT[64:65, 128 * j : 128 * (j + 1)],
                        ident_f[64:65, 64:65],
                    )
                rc = rpool.tile([128, 4], F32, name="rc")
                nc.vector.reciprocal(rc[:], dp[:])
                for j in range(4):
                    ot2 = ps_t.tile([128, 64], F32, name="ps_ot2")
                    nc.tensor.transpose(
                        ot2[:],
                        oT[0:64, 128 * j : 128 * (j + 1)],
                        ident_f[0:64, 0:64],
                    )
                    col = 128 * (4 * qc + j) + 64 * hh
                    nc.vector.tensor_scalar_mul(
                        outsb[b][:, col : col + 64], ot2[:], rc[:, j : j + 1]
                    )

    for b in range(B):
        dst = out_ap[2048 * b : 2048 * (b + 1), :].rearrange("(k p) c -> p k c", p=128)
        src = outsb[b][:].rearrange("p (k c) -> p k c", c=128)
        nc.sync.dma_start(dst, src)


def build_program():
    nc = bacc.Bacc("TRN2", target_bir_lowering=False, debug=False, num_devices=NCORES)
    xT_h = nc.dram_tensor("xT", [C, TOK], BF16, kind="ExternalInput").ap()
    w_h = nc.dram_tensor("w", [C, N3], BF16, kind="ExternalInput").ap()
    b_h = nc.dram_tensor("b", [128, 19], F32, kind="ExternalInput").ap()
    out_h = nc.dram_tensor("out", [TOK, CS], F32, kind="ExternalOutput").ap()
    with tile.TileContext(nc) as tc:
        mha_kernel(tc, out_h, xT_h, w_h, b_h)
    nc.compile()
    return nc


def make_in_maps(x, w_qkv, b_qkv):
    x = np.asarray(x, dtype=np.float32)
    w_qkv = np.asarray(w_qkv, dtype=np.float32)
    b_qkv = np.asarray(b_qkv, dtype=np.float32)
    xT = np.ascontiguousarray(x.reshape(TOK, C).T).astype(ml_dtypes.bfloat16)
    in_maps = []
    for c in range(NCORES):
        sl = slice(CS * c, CS * (c + 1))
        w_c = np.ascontiguousarray(
            np.concatenate(
                [w_qkv[:, sl], w_qkv[:, C + CS * c : C + CS * (c + 1)],
                 w_qkv[:, 2 * C + CS * c : 2 * C + CS * (c + 1)]],
                axis=1,
            )
        ).astype(ml_dtypes.bfloat16)
        b_c = np.concatenate(
            [b_qkv[sl], b_qkv[C + CS * c : C + CS * (c + 1)],
             b_qkv[2 * C + CS * c : 2 * C + CS * (c + 1)]]
        )
        b_c = b_c.reshape(3, 128).T  # [128, 3]
        b_c = np.ascontiguousarray(
            np.concatenate([b_c, np.ones((128, 16), np.float32)], axis=1)
        )  # [128, 19]
        in_maps.append({"xT": xT, "w": w_c, "b": b_c})
    return in_maps


_NC_CACHE = None


def kernel(x, w_qkv, b_qkv):
    global _NC_CACHE
    if _NC_CACHE is None:
        _NC_CACHE = build_program()
    nc = _NC_CACHE
    in_maps = make_in_maps(x, w_qkv, b_qkv)
    res = run_bass_kernel_spmd(nc, in_maps, list(range(NCORES)))
    outs = [res.results[c]["out"].reshape(B, T, CS) for c in range(NCORES)]
    return np.concatenate(outs, axis=2)


if __name__ == "__main__":
    rng = np.random.default_rng(0)
    x = rng.standard_normal((B, T, C), dtype=np.float32)
    w = (rng.standard_normal((C, 3 * C), dtype=np.float32) / np.sqrt(C)).astype(np.float32)
    bq = (rng.standard_normal((3 * C,), dtype=np.float32) * 0.02).astype(np.float32)
    out = kernel(x, w, bq)
    print("out", out.shape, out.dtype)


# revision 11
# speedup vs baseline: 72.3713x; 1.3622x over previous
"""Multi-head causal attention (B=2, T=2048, C=1024, H=16, D=64) on 8 TRN2
NeuronCores, tensor-parallel over heads: core c owns heads {2c, 2c+1}.

Per-core program (SPMD, same code, per-core weight slices), all matmuls bf16
with fp32 PSUM accumulation:
  qkv^T = W_c^T @ x^T + b_c      [384, 4096]
  K^T repacked zero-padded to K=128 per head (kpad0 = [K_h0; 0],
    kpad1 = [0; K_h1]) so score matmuls use the full 128x128 array.
  V natural [k, d] via PE transpose, plus a ones column per k-chunk.
  per 512-wide q-chunk x (b, head)  [interleaved for PE density / HAM warmth]:
    s^T[k, q] matmuls (two k-tiles share a 2-bank psum -> one 1024-wide exp),
    causal diagonal tiles get range-limited exp + gpsimd affine_select zeroing,
    outT[d,q] + denom row = [V | 1].T @ P^T, M=128 overlapping lhsT windows,
    PE-transpose back to [q, d] (bf16), scale by 1/denom on DVE, assemble
    the [q, 128] channel slice, one big DMA out per batch.
Host: transposes x, converts x/W to bf16 (input layout prep), slices W/b per
core, concatenates per-core [4096, 128] outputs on channels.
"""
import sys

if "/opt/trn_rl_repo" not in sys.path:
    sys.path.insert(0, "/opt/trn_rl_repo")

from contextlib import ExitStack

import numpy as np
import ml_dtypes

import concourse.bass as bass
import concourse.tile as tile
from concourse import bacc, mybir
from concourse._compat import with_exitstack
from concourse.bass_utils import run_bass_kernel_spmd
from concourse.masks import make_identity

F32 = mybir.dt.float32
BF16 = mybir.dt.bfloat16
EXPF = mybir.ActivationFunctionType.Exp

B, T, C = 2, 2048, 1024
H, D = 16, 64
NCORES = 8
TOK = B * T            # 4096
CS = 128               # channel slice per core (2 heads x 64)
N3 = 3 * CS            # 384 qkv columns per core
SCALE = 1.0 / np.sqrt(D)


@with_exitstack
def mha_kernel(ctx: ExitStack, tc: tile.TileContext, out_ap, xT_ap, w_ap, b_ap):
    nc = tc.nc

    cst = ctx.enter_context(tc.tile_pool(name="cst", bufs=1))
    xpool = ctx.enter_context(tc.tile_pool(name="xt", bufs=10))
    pbig = ctx.enter_context(tc.tile_pool(name="pbig", bufs=10))
    pdiag = ctx.enter_context(tc.tile_pool(name="pdiag", bufs=8))
    otpool = ctx.enter_context(tc.tile_pool(name="ot", bufs=3))
    rpool = ctx.enter_context(tc.tile_pool(name="rc", bufs=3))
    psA = ctx.enter_context(tc.tile_pool(name="psA", bufs=2, space="PSUM"))  # big: proj + paired scores
    psB = ctx.enter_context(tc.tile_pool(name="psB", bufs=2, space="PSUM"))  # diag scores
    psC = ctx.enter_context(tc.tile_pool(name="psC", bufs=1, space="PSUM"))  # pv
    psD = ctx.enter_context(tc.tile_pool(name="psD", bufs=1, space="PSUM"))  # transposes

    ident_f = cst.tile([128, 128], F32, name="ident_f")
    make_identity(nc, ident_f[:])
    ident_b = cst.tile([128, 128], BF16, name="ident_b")
    nc.vector.tensor_copy(ident_b[:], ident_f[:])

    # bias cols 0-2; cols 3-18 are ones (for the PV denominator column)
    bias = cst.tile([128, 19], F32, name="bias")
    nc.sync.dma_start(bias[:], b_ap[:])

    W8 = []
    for cc in range(8):
        w = cst.tile([128, N3], BF16, name=f"w{cc}")
        nc.sync.dma_start(w[:], w_ap[128 * cc : 128 * (cc + 1), :])
        W8.append(w)

    qkvT = [cst.tile([128, TOK], BF16, name=f"qkvT{nt}") for nt in range(3)]

    # ---- projection: qkvT[nt][:, t0:t0+512] = W[:, nt].T @ xT[:, t] + b ----
    for tq in range(4):  # t-quarters of 1024 tokens
        xts = []
        for cc in range(8):
            xt = xpool.tile([128, 1024], BF16, name="xt")
            nc.sync.dma_start(
                xt[:],
                xT_ap[128 * cc : 128 * (cc + 1), 1024 * tq : 1024 * (tq + 1)],
            )
            xts.append(xt)
        for th in range(2):
            t0 = 1024 * tq + 512 * th
            for nt in range(3):
                ps = psA.tile([128, 1024], F32, name="psA")
                for cc in range(8):
                    nc.tensor.matmul(
                        ps[:, 0:512],
                        lhsT=W8[cc][:, 128 * nt : 128 * (nt + 1)],
                        rhs=xts[cc][:, 512 * th : 512 * (th + 1)],
                        start=(cc == 0),
                        stop=(cc == 7),
                    )
                nc.vector.tensor_scalar_add(
                    qkvT[nt][:, t0 : t0 + 512], ps[:, 0:512], bias[:, nt : nt + 1]
                )

    # ---- zero-padded K^T per (b, head): full-K score matmuls ----
    # kpad[b][0] rows 0-63 = K^T_h0, rows 64-127 = 0
    # kpad[b][1] rows 0-63 = 0,      rows 64-127 = K^T_h1
    kpad = []
    for b in range(B):
        k0t = cst.tile([128, 2048], BF16, name=f"kpad0_{b}")
        nc.vector.tensor_copy(k0t[0:64, :], qkvT[1][0:64, 2048 * b : 2048 * (b + 1)])
        nc.vector.memset(k0t[64:128, :], 0.0)
        k1t = cst.tile([128, 2048], BF16, name=f"kpad1_{b}")
        nc.vector.memset(k1t[0:64, :], 0.0)
        nc.vector.tensor_copy(k1t[64:128, :], qkvT[1][64:128, 2048 * b : 2048 * (b + 1)])
        kpad.append((k0t, k1t))

    # ---- V natural [k, d] + ones column, per (b, hh) ----
    V1 = []
    for b in range(B):
        for hh in range(2):
            v1 = cst.tile([128, 16 * 65], BF16, name=f"v1_{b}_{hh}")
            v3 = v1[:].rearrange("p (k u) -> p k u", u=65)
            nc.vector.tensor_copy(v3[:, :, 64], bias[:, 3:19])
            for g in range(2):
                tp = psD.tile([128, 512], BF16, name="psD", tag="psD")
                for j in range(8):
                    kc = 8 * g + j
                    nc.tensor.transpose(
                        tp[:, 64 * j : 64 * (j + 1)],
                        qkvT[2][64 * hh : 64 * (hh + 1), 2048 * b + 128 * kc : 2048 * b + 128 * (kc + 1)],
                        ident_b[64 * hh : 64 * hh + 64, 64 * hh : 64 * hh + 64],
                    )
                nc.vector.tensor_copy(
                    v3[:, 8 * g : 8 * (g + 1), 0:64],
                    tp[:].rearrange("p (k u) -> p k u", u=64),
                )
            V1.append(v1)

    # ---- attention: qc outer, (b, hh) inner for PE density ----
    outsb = [cst.tile([128, 2048], F32, name=f"outsb{b}") for b in range(B)]

    for qc in range(4):
        for b in range(B):
            for hh in range(2):
                q0 = 2048 * b + 512 * qc
                nk = 4 * qc + 4
                kh = kpad[b][hh]
                qh = qkvT[0][:, q0 : q0 + 512]
                v1 = V1[2 * b + hh]

                # paired full k-tiles -> [128, 1024] psum -> one wide exp
                rhs_slices = []  # (P tile, col offset) per ki
                for kj in range(2 * qc):
                    spb = psA.tile([128, 1024], F32, name="psA")
                    for half in range(2):
                        ki = 2 * kj + half
                        nc.tensor.matmul(
                            spb[:, 512 * half : 512 * (half + 1)],
                            lhsT=kh[:, 128 * ki : 128 * (ki + 1)],
                            rhs=qh,
                            start=True,
                            stop=True,
                        )
                    Pb = pbig.tile([128, 1024], BF16, name="Pbig")
                    nc.scalar.activation(Pb[:], spb[:], EXPF, scale=SCALE)
                    rhs_slices.append((Pb, 0))
                    rhs_slices.append((Pb, 512))
                # diagonal k-tiles
                for ki in range(4 * qc, nk):
                    spd = psB.tile([128, 512], F32, name="psB")
                    nc.tensor.matmul(
                        spd[:],
                        lhsT=kh[:, 128 * ki : 128 * (ki + 1)],
                        rhs=qh,
                        start=True,
                        stop=True,
                    )
                    Pd = pdiag.tile([128, 512], BF16, name="Pdiag")
                    r = 128 * ki - 512 * qc
                    nc.scalar.activation(Pd[:, r:512], spd[:, r:512], EXPF, scale=SCALE)
                    nc.gpsimd.affine_select(
                        out=Pd[:, r:512],
                        in_=Pd[:, r:512],
                        compare_op=mybir.AluOpType.is_ge,
                        fill=0.0,
                        base=0,
                        pattern=[[1, 512 - r]],
                        channel_multiplier=-1,
                    )
                    rhs_slices.append((Pd, r - 512))  # negative marks diag offset r

                op = psC.tile([128, 512], F32, name="psC")
                for ki in range(nk):
                    Pt, off = rhs_slices[ki]
                    if off >= 0:
                        rhs = Pt[:, off : off + 512]
                        r = 0
                    else:
                        r = off + 512
                        rhs = Pt[:, r:512]
                    m = 128 if ki < 15 else 65
                    nc.tensor.matmul(
                        op[0:m, r:512],
                        lhsT=v1[:, 65 * ki : 65 * ki + m],
                        rhs=rhs,
                        start=(ki == 0),
                        stop=(ki == nk - 1),
                        skip_group_check=True,
                    )
                oT = otpool.tile([65, 512], BF16, name="oT")
                nc.vector.tensor_copy(oT[:], op[0:65, :])

                tr = psD.tile([128, 272], BF16, name="psD_tr", tag="psD")
                for j in range(4):
                    nc.tensor.transpose(
                        tr[:, 256 + 4 * j : 257 + 4 * j],
                        oT[64:65, 128 * j : 128 * (j + 1)],
                        ident_b[64:65, 64:65],
                    )
                    nc.tensor.transpose(
                        tr[:, 64 * j : 64 * (j + 1)],
                        oT[0:64, 128 * j : 128 * (j + 1)],
                        ident_b[0:64, 0:64],
                    )
                rc = rpool.tile([128, 4], F32, name="rc")
                den4 = tr[:, 256:272].rearrange("p (a b) -> p a b", b=4)[:, :, 0]
                nc.vector.reciprocal(rc[:], den4)
                for j in range(4):
                    col = 128 * (4 * qc + j) + 64 * hh
                    nc.vector.tensor_scalar_mul(
                        outsb[b][:, col : col + 64],
                        tr[:, 64 * j : 64 * (j + 1)],
                        rc[:, j : j + 1],
                    )

    for b in range(B):
        dst = out_ap[2048 * b : 2048 * (b + 1), :].rearrange("(k p) c -> p k c", p=128)
        src = outsb[b][:].rearrange("p (k c) -> p k c", c=128)
        nc.sync.dma_start(dst, src)


def build_program():
    nc = bacc.Bacc("TRN2", target_bir_lowering=False, debug=False, num_devices=NCORES)
    xT_h = nc.dram_tensor("xT", [C, TOK], BF16, kind="ExternalInput").ap()
    w_h = nc.dram_tensor("w", [C, N3], BF16, kind="ExternalInput").ap()
    b_h = nc.dram_tensor("b", [128, 19], F32, kind="ExternalInput").ap()
    out_h = nc.dram_tensor("out", [TOK, CS], F32, kind="ExternalOutput").ap()
    with tile.TileContext(nc) as tc:
        mha_kernel(tc, out_h, xT_h, w_h, b_h)
    nc.compile()
    return nc


def make_in_maps(x, w_qkv, b_qkv):
    x = np.asarray(x, dtype=np.float32)
    w_qkv = np.asarray(w_qkv, dtype=np.float32)
    b_qkv = np.asarray(b_qkv, dtype=np.float32)
    xT = np.ascontiguousarray(x.reshape(TOK, C).T).astype(ml_dtypes.bfloat16)
    in_maps = []
    for c in range(NCORES):
        sl = slice(CS * c, CS * (c + 1))
        w_c = np.ascontiguousarray(
            np.concatenate(
                [w_qkv[:, sl], w_qkv[:, C + CS * c : C + CS * (c + 1)],
                 w_qkv[:, 2 * C + CS * c : 2 * C + CS * (c + 1)]],
                axis=1,
            )
        ).astype(ml_dtypes.bfloat16)
        b_c = np.concatenate(
            [b_qkv[sl], b_qkv[C + CS * c : C + CS * (c + 1)],
             b_qkv[2 * C + CS * c : 2 * C + CS * (c + 1)]]
        )
        b_c = b_c.reshape(3, 128).T  # [128, 3]
        b_c = np.ascontiguousarray(
            np.concatenate([b_c, np.ones((128, 16), np.float32)], axis=1)
        )  # [128, 19]
        in_maps.append({"xT": xT, "w": w_c, "b": b_c})
    return in_maps


_NC_CACHE = None


def kernel(x, w_qkv, b_qkv):
    global _NC_CACHE
    if _NC_CACHE is None:
        _NC_CACHE = build_program()
    nc = _NC_CACHE
    in_maps = make_in_maps(x, w_qkv, b_qkv)
    res = run_bass_kernel_spmd(nc, in_maps, list(range(NCORES)))
    outs = [res.results[c]["out"].reshape(B, T, CS) for c in range(NCORES)]
    return np.concatenate(outs, axis=2)


if __name__ == "__main__":
    rng = np.random.default_rng(0)
    x = rng.standard_normal((B, T, C), dtype=np.float32)
    w = (rng.standard_normal((C, 3 * C), dtype=np.float32) / np.sqrt(C)).astype(np.float32)
    bq = (rng.standard_normal((3 * C,), dtype=np.float32) * 0.02).astype(np.float32)
    out = kernel(x, w, bq)
    print("out", out.shape, out.dtype)


# revision 12
# speedup vs baseline: 77.6608x; 1.0731x over previous
"""Multi-head causal attention (B=2, T=2048, C=1024, H=16, D=64) on 8 TRN2
NeuronCores, tensor-parallel over heads: core c owns heads {2c, 2c+1}.

Per-core program (SPMD, same code, per-core weight slices), all matmuls bf16
with fp32 PSUM accumulation. Pipelined in 4 stages of 1024 tokens each:
projection chunk -> V-transposes + zero-padded K^T piece -> the attention
q-chunks whose causal window is now complete. This keeps TensorE dense
(HAM stays at full clock) and lets ScalarE exp overlap the next projection.

Attention per (b, head, 512-wide q-chunk):
  s^T[k, q] = Kpad^T.T @ Q^T  (K=128 via zero-padding, full-array matmuls;
  two k-tiles share a 2-bank psum -> one 1024-wide exp), causal diagonal
  tiles get range-limited exp + gpsimd affine_select zeroing,
  outT[d,q] + denom row = [V | 1].T @ P^T with M=128 overlapping lhsT
  windows, PE-transpose back to [q, d] (bf16), scale by 1/denom on DVE.
Host: transposes x, converts x/W to bf16, slices W/b per core, concatenates
per-core [4096, 128] outputs on channels.
"""
import sys

if "/opt/trn_rl_repo" not in sys.path:
    sys.path.insert(0, "/opt/trn_rl_repo")

from contextlib import ExitStack

import numpy as np
import ml_dtypes

import concourse.bass as bass
import concourse.tile as tile
from concourse import bacc, mybir
from concourse._compat import with_exitstack
from concourse.bass_utils import run_bass_kernel_spmd
from concourse.masks import make_identity

F32 = mybir.dt.float32
BF16 = mybir.dt.bfloat16
EXPF = mybir.ActivationFunctionType.Exp

B, T, C = 2, 2048, 1024
H, D = 16, 64
NCORES = 8
TOK = B * T            # 4096
CS = 128               # channel slice per core (2 heads x 64)
N3 = 3 * CS            # 384 qkv columns per core
SCALE = 1.0 / np.sqrt(D)


@with_exitstack
def mha_kernel(ctx: ExitStack, tc: tile.TileContext, out_ap, xT_ap, w_ap, b_ap):
    nc = tc.nc

    cst = ctx.enter_context(tc.tile_pool(name="cst", bufs=1))
    xpool = ctx.enter_context(tc.tile_pool(name="xt", bufs=10))
    pbig = ctx.enter_context(tc.tile_pool(name="pbig", bufs=10))
    pdiag = ctx.enter_context(tc.tile_pool(name="pdiag", bufs=8))
    otpool = ctx.enter_context(tc.tile_pool(name="ot", bufs=3))
    rpool = ctx.enter_context(tc.tile_pool(name="rc", bufs=3))
    psA = ctx.enter_context(tc.tile_pool(name="psA", bufs=2, space="PSUM"))  # big: proj + paired scores
    psB = ctx.enter_context(tc.tile_pool(name="psB", bufs=2, space="PSUM"))  # diag scores
    psC = ctx.enter_context(tc.tile_pool(name="psC", bufs=1, space="PSUM"))  # pv
    psD = ctx.enter_context(tc.tile_pool(name="psD", bufs=1, space="PSUM"))  # transposes

    ident_f = cst.tile([128, 128], F32, name="ident_f")
    make_identity(nc, ident_f[:])
    ident_b = cst.tile([128, 128], BF16, name="ident_b")
    nc.vector.tensor_copy(ident_b[:], ident_f[:])

    # bias cols 0-2; cols 3-18 are ones (for the PV denominator column)
    bias = cst.tile([128, 19], F32, name="bias")
    nc.sync.dma_start(bias[:], b_ap[:])

    W8 = []
    for cc in range(8):
        w = cst.tile([128, N3], BF16, name=f"w{cc}")
        nc.sync.dma_start(w[:], w_ap[128 * cc : 128 * (cc + 1), :])
        W8.append(w)

    qkvT = [cst.tile([128, TOK], BF16, name=f"qkvT{nt}") for nt in range(3)]

    # persistent attention tensors
    # kpad[b][0]: rows 0-63 = K^T_h0, rows 64-127 = 0
    # kpad[b][1]: rows 0-63 = 0,      rows 64-127 = K^T_h1
    kpad = []
    for b in range(B):
        k0t = cst.tile([128, 2048], BF16, name=f"kpad0_{b}")
        nc.vector.memset(k0t[64:128, :], 0.0)
        k1t = cst.tile([128, 2048], BF16, name=f"kpad1_{b}")
        nc.vector.memset(k1t[0:64, :], 0.0)
        kpad.append((k0t, k1t))
    V1 = []
    for b in range(B):
        for hh in range(2):
            v1 = cst.tile([128, 16 * 65], BF16, name=f"v1_{b}_{hh}")
            v3 = v1[:].rearrange("p (k u) -> p k u", u=65)
            nc.vector.tensor_copy(v3[:, :, 64], bias[:, 3:19])
            V1.append(v1)
    outsb = [cst.tile([128, 2048], F32, name=f"outsb{b}") for b in range(B)]

    def attention_unit(b, hh, qc):
        q0 = 2048 * b + 512 * qc
        nk = 4 * qc + 4
        kh = kpad[b][hh]
        qh = qkvT[0][:, q0 : q0 + 512]
        v1 = V1[2 * b + hh]

        # paired full k-tiles -> [128, 1024] psum -> one wide exp
        rhs_slices = []  # (P tile, col offset or diag marker) per ki
        for kj in range(2 * qc):
            spb = psA.tile([128, 1024], F32, name="psA")
            for half in range(2):
                ki = 2 * kj + half
                nc.tensor.matmul(
                    spb[:, 512 * half : 512 * (half + 1)],
                    lhsT=kh[:, 128 * ki : 128 * (ki + 1)],
                    rhs=qh,
                    start=True,
                    stop=True,
                )
            Pb = pbig.tile([128, 1024], BF16, name="Pbig")
            nc.scalar.activation(Pb[:], spb[:], EXPF, scale=SCALE)
            rhs_slices.append((Pb, 0))
            rhs_slices.append((Pb, 512))
        # diagonal k-tiles
        for ki in range(4 * qc, nk):
            spd = psB.tile([128, 512], F32, name="psB")
            nc.tensor.matmul(
                spd[:],
                lhsT=kh[:, 128 * ki : 128 * (ki + 1)],
                rhs=qh,
                start=True,
                stop=True,
            )
            Pd = pdiag.tile([128, 512], BF16, name="Pdiag")
            r = 128 * ki - 512 * qc
            nc.scalar.activation(Pd[:, r:512], spd[:, r:512], EXPF, scale=SCALE)
            nc.gpsimd.affine_select(
                out=Pd[:, r:512],
                in_=Pd[:, r:512],
                compare_op=mybir.AluOpType.is_ge,
                fill=0.0,
                base=0,
                pattern=[[1, 512 - r]],
                channel_multiplier=-1,
            )
            rhs_slices.append((Pd, r - 512))  # negative marks diag offset r

        op = psC.tile([128, 512], F32, name="psC")
        for ki in range(nk):
            Pt, off = rhs_slices[ki]
            if off >= 0:
                rhs = Pt[:, off : off + 512]
                r = 0
            else:
                r = off + 512
                rhs = Pt[:, r:512]
            m = 128 if ki < 15 else 65
            nc.tensor.matmul(
                op[0:m, r:512],
                lhsT=v1[:, 65 * ki : 65 * ki + m],
                rhs=rhs,
                start=(ki == 0),
                stop=(ki == nk - 1),
                skip_group_check=True,
            )
        oT = otpool.tile([65, 512], BF16, name="oT")
        nc.vector.tensor_copy(oT[:], op[0:65, :])

        tr = psD.tile([128, 272], BF16, name="psD_tr", tag="psD")
        for j in range(4):
            nc.tensor.transpose(
                tr[:, 256 + 4 * j : 257 + 4 * j],
                oT[64:65, 128 * j : 128 * (j + 1)],
                ident_b[64:65, 64:65],
            )
            nc.tensor.transpose(
                tr[:, 64 * j : 64 * (j + 1)],
                oT[0:64, 128 * j : 128 * (j + 1)],
                ident_b[0:64, 0:64],
            )
        rc = rpool.tile([128, 4], F32, name="rc")
        den4 = tr[:, 256:272].rearrange("p (a b) -> p a b", b=4)[:, :, 0]
        nc.vector.reciprocal(rc[:], den4)
        for j in range(4):
            col = 128 * (4 * qc + j) + 64 * hh
            nc.vector.tensor_scalar_mul(
                outsb[b][:, col : col + 64],
                tr[:, 64 * j : 64 * (j + 1)],
                rc[:, j : j + 1],
            )

    # ---- pipelined stages: proj chunk -> K/V prep piece -> ready q-chunks ----
    for tq in range(4):  # 1024-token stages; b = tq // 2, piece g = tq % 2
        b, g = tq // 2, tq % 2
        xts = []
        for cc in range(8):
            xt = xpool.tile([128, 1024], BF16, name="xt")
            nc.sync.dma_start(
                xt[:],
                xT_ap[128 * cc : 128 * (cc + 1), 1024 * tq : 1024 * (tq + 1)],
            )
            xts.append(xt)
        for th in range(2):
            t0 = 1024 * tq + 512 * th
            for nt in range(3):
                ps = psA.tile([128, 1024], F32, name="psA")
                for cc in range(8):
                    nc.tensor.matmul(
                        ps[:, 0:512],
                        lhsT=W8[cc][:, 128 * nt : 128 * (nt + 1)],
                        rhs=xts[cc][:, 512 * th : 512 * (th + 1)],
                        start=(cc == 0),
                        stop=(cc == 7),
                    )
                nc.vector.tensor_scalar_add(
                    qkvT[nt][:, t0 : t0 + 512], ps[:, 0:512], bias[:, nt : nt + 1]
                )

        # K^T zero-padded piece (this stage's 1024 tokens)
        lo = 1024 * g
        nc.vector.tensor_copy(
            kpad[b][0][0:64, lo : lo + 1024],
            qkvT[1][0:64, 1024 * tq : 1024 * (tq + 1)],
        )
        nc.vector.tensor_copy(
            kpad[b][1][64:128, lo : lo + 1024],
            qkvT[1][64:128, 1024 * tq : 1024 * (tq + 1)],
        )
        # V natural piece: k-chunks 8g .. 8g+7
        for hh in range(2):
            v3 = V1[2 * b + hh][:].rearrange("p (k u) -> p k u", u=65)
            tp = psD.tile([128, 512], BF16, name="psD_vt", tag="psD")
            for j in range(8):
                kc = 8 * g + j
                nc.tensor.transpose(
                    tp[:, 64 * j : 64 * (j + 1)],
                    qkvT[2][64 * hh : 64 * (hh + 1), 2048 * b + 128 * kc : 2048 * b + 128 * (kc + 1)],
                    ident_b[64 * hh : 64 * hh + 64, 64 * hh : 64 * hh + 64],
                )
            nc.vector.tensor_copy(
                v3[:, 8 * g : 8 * (g + 1), 0:64],
                tp[:].rearrange("p (k u) -> p k u", u=64),
            )

        # attention q-chunks now complete: qc in {2g, 2g+1}
        for qc in (2 * g, 2 * g + 1):
            for hh in range(2):
                attention_unit(b, hh, qc)

        if g == 1:  # batch b fully done
            dst = out_ap[2048 * b : 2048 * (b + 1), :].rearrange("(k p) c -> p k c", p=128)
            src = outsb[b][:].rearrange("p (k c) -> p k c", c=128)
            nc.sync.dma_start(dst, src)


def build_program():
    nc = bacc.Bacc("TRN2", target_bir_lowering=False, debug=False, num_devices=NCORES)
    xT_h = nc.dram_tensor("xT", [C, TOK], BF16, kind="ExternalInput").ap()
    w_h = nc.dram_tensor("w", [C, N3], BF16, kind="ExternalInput").ap()
    b_h = nc.dram_tensor("b", [128, 19], F32, kind="ExternalInput").ap()
    out_h = nc.dram_tensor("out", [TOK, CS], F32, kind="ExternalOutput").ap()
    with tile.TileContext(nc) as tc:
        mha_kernel(tc, out_h, xT_h, w_h, b_h)
    nc.compile()
    return nc


def make_in_maps(x, w_qkv, b_qkv):
    x = np.asarray(x, dtype=np.float32)
    w_qkv = np.asarray(w_qkv, dtype=np.float32)
    b_qkv = np.asarray(b_qkv, dtype=np.float32)
    xT = np.ascontiguousarray(x.reshape(TOK, C).T).astype(ml_dtypes.bfloat16)
    in_maps = []
    for c in range(NCORES):
        sl = slice(CS * c, CS * (c + 1))
        w_c = np.ascontiguousarray(
            np.concatenate(
                [w_qkv[:, sl], w_qkv[:, C + CS * c : C + CS * (c + 1)],
                 w_qkv[:, 2 * C + CS * c : 2 * C + CS * (c + 1)]],
                axis=1,
            )
        ).astype(ml_dtypes.bfloat16)
        b_c = np.concatenate(
            [b_qkv[sl], b_qkv[C + CS * c : C + CS * (c + 1)],
             b_qkv[2 * C + CS * c : 2 * C + CS * (c + 1)]]
        )
        b_c = b_c.reshape(3, 128).T  # [128, 3]
        b_c = np.ascontiguousarray(
            np.concatenate([b_c, np.ones((128, 16), np.float32)], axis=1)
        )  # [128, 19]
        in_maps.append({"xT": xT, "w": w_c, "b": b_c})
    return in_maps


_NC_CACHE = None


def kernel(x, w_qkv, b_qkv):
    global _NC_CACHE
    if _NC_CACHE is None:
        _NC_CACHE = build_program()
    nc = _NC_CACHE
    in_maps = make_in_maps(x, w_qkv, b_qkv)
    res = run_bass_kernel_spmd(nc, in_maps, list(range(NCORES)))
    outs = [res.results[c]["out"].reshape(B, T, CS) for c in range(NCORES)]
    return np.concatenate(outs, axis=2)


if __name__ == "__main__":
    rng = np.random.default_rng(0)
    x = rng.standard_normal((B, T, C), dtype=np.float32)
    w = (rng.standard_normal((C, 3 * C), dtype=np.float32) / np.sqrt(C)).astype(np.float32)
    bq = (rng.standard_normal((3 * C,), dtype=np.float32) * 0.02).astype(np.float32)
    out = kernel(x, w, bq)
    print("out", out.shape, out.dtype)
